# revision 1
# baseline (speedup 1.0000x reference)
"""Trainium2 Bass kernel for nn_CAL_51015621542567.

Cross-attention (D queries over T keys, L features) + gated residual +
LayerNorm(D) + ReLU + BatchNorm2d(train) + 1x1 conv + LayerNorm(D) + gate.

Data-parallel over batch: B=32 sharded as 4 batches on each of 8 NeuronCores.
Two NEFF launches: program 1 computes attention + LN + ReLU and per-core
BatchNorm partial stats; the host sums the 8 cores' [2,768] partials (the
cross-device all-reduce); program 2 applies BN (folded into the 1x1-conv
weights), the conv, post-LN and the gate.

All big matmuls run as float32r (measured bit-identical to fp32 on TRN2 HW
at ~1 cycle/row for free dim >= 256; fp32 mode is 4 cycles/row).
"""

import os
import sys

sys.path.insert(0, "/opt/trn_rl_repo")

from contextlib import ExitStack

import numpy as np
import concourse.bass as bass
from concourse import bacc
import concourse.mybir as mybir
import concourse.tile as tile
from concourse.masks import make_identity
from concourse.bass_utils import run_bass_kernel_spmd

AF = mybir.ActivationFunctionType
OP = mybir.AluOpType

P = 128
B, D, T, L = 32, 768, 768, 1024
NCORES = int(os.environ.get("K_NC", "8"))
BL = int(os.environ.get("K_BL", str(B // 8)))  # batches per core
PHASES = os.environ.get("K_PHASES", "full")
DO_STATS = PHASES in ("C2", "C3", "full")
DO_RELU_ACC = PHASES in ("C3", "full")
DO_TTR = PHASES == "full"
DT, TT, LT, OT = D // P, T // P, L // P, D // P
EPS = 1e-5
NB = float(B * L)  # BatchNorm stat count

f32 = mybir.dt.float32
f32r = mybir.dt.float32r

N1 = 384  # mm1 free-dim chunk (2 x 384, each within one PSUM bank)
N2 = 512  # mm2/mm3 free-dim chunk over L (2 x 512)


def _common_pools(st, tc):
    return dict(
        cp=st.enter_context(tc.tile_pool(name="consts", bufs=1)),
        bigp=st.enter_context(tc.tile_pool(name="big", bufs=1)),
        sqp=st.enter_context(tc.tile_pool(name="sqpool", bufs=2)),
        smallp=st.enter_context(tc.tile_pool(name="small", bufs=4)),
        rowp=st.enter_context(tc.tile_pool(name="rows", bufs=1)),
        psp=st.enter_context(tc.tile_pool(name="psum", bufs=1, space="PSUM")),
    )


def _consts(nc, cp):
    ident_f = cp.tile([P, P], f32, name="ident_f", tag="ident_f")
    make_identity(nc, ident_f[:])
    ident = cp.tile([P, P], f32r, name="ident", tag="ident")
    nc.vector.tensor_copy(ident[:], ident_f[:])

    ones_col_f = cp.tile([P, 1], f32, name="ones_col_f", tag="ones_col_f")
    nc.gpsimd.memset(ones_col_f[:], 1.0)
    ones_col = cp.tile([P, 1], f32r, name="ones_col", tag="ones_col")
    nc.vector.tensor_copy(ones_col[:], ones_col_f[:])

    ones_row_f = cp.tile([1, P], f32, name="ones_row_f", tag="ones_row_f")
    nc.gpsimd.memset(ones_row_f[:], 1.0)
    ones_row = cp.tile([1, P], f32r, name="ones_row", tag="ones_row")
    nc.vector.tensor_copy(ones_row[:], ones_row_f[:])

    eps_col = cp.tile([P, 1], f32, name="eps_col", tag="eps_col")
    nc.gpsimd.memset(eps_col[:], EPS)
    return ident, ones_col, ones_row, eps_col


def _ln_rows(nc, rowp, psSum, psSq, eps_col):
    """Finalize per-column LayerNorm stats from [1,2,512] psum sums.

    Returns (mu_row, rs_row) as [1, L] f32r rows."""
    mu_row = rowp.tile([1, L], f32r, tag="mu_row")
    nc.vector.tensor_scalar_mul(
        mu_row[:], psSum[:1].rearrange("p c n -> p (c n)"), 1.0 / D
    )
    m2_row = rowp.tile([1, L], f32, tag="m2_row")
    nc.vector.tensor_tensor(
        m2_row[:], mu_row[:].bitcast(f32), mu_row[:].bitcast(f32), OP.mult
    )
    v_row = rowp.tile([1, L], f32, tag="v_row")
    nc.vector.tensor_scalar_mul(
        v_row[:], psSq[:1].rearrange("p c n -> p (c n)"), 1.0 / D
    )
    nc.vector.tensor_tensor(v_row[:], v_row[:], m2_row[:], OP.subtract)
    l_row = rowp.tile([1, L], f32, tag="m2_row")
    nc.scalar.activation(l_row[:], v_row[:], AF.Ln, bias=eps_col[:1, :])
    rs_row = rowp.tile([1, L], f32r, tag="rs_row")
    nc.scalar.activation(rs_row[:], l_row[:], AF.Exp, scale=-0.5)
    return mu_row, rs_row


def _bcast_rows(nc, psp, ones_row, mu_row, rs_row):
    """Broadcast [1, L] stat rows to [P, L] via K=1 matmuls into psum."""
    psBmu = psp.tile([P, 2, 512], f32, tag="ps2b", bufs=2, name="psBmu")
    psBrs = psp.tile([P, 2, 512], f32, tag="ps2b", bufs=2, name="psBrs")
    for ch in range(2):
        csl = slice(ch * N2, (ch + 1) * N2)
        nc.tensor.matmul(
            psBmu[:, ch, :], ones_row[:], mu_row[:, csl], start=True, stop=True
        )
        nc.tensor.matmul(
            psBrs[:, ch, :], ones_row[:], rs_row[:, csl], start=True, stop=True
        )
    return psBmu, psBrs


# ===================== program 1: attention + LN + ReLU =====================

def build_program1():
    nc = bacc.Bacc("TRN2", target_bir_lowering=False, debug=False,
                   num_devices=NCORES)
    x_d = nc.dram_tensor("x", [BL, D, L], f32, kind="ExternalInput").ap()
    y_d = nc.dram_tensor("y", [BL, T, L], f32, kind="ExternalInput").ap()
    gav_d = nc.dram_tensor("gate_av", [1], f32, kind="ExternalInput").ap()
    lnw_d = nc.dram_tensor("ln_before_w", [D], f32, kind="ExternalInput").ap()
    lnb_d = nc.dram_tensor("ln_before_b", [D], f32, kind="ExternalInput").ap()
    hr_d = nc.dram_tensor("hr", [BL, D, L], f32, kind="ExternalOutput").ap()
    st_d = nc.dram_tensor("bn_part", [2, D], f32, kind="ExternalOutput").ap()

    with tile.TileContext(nc) as tc:
        with ExitStack() as st:
            p = _common_pools(st, tc)
            cp, bigp, sqp, smallp, rowp, psp = (
                p["cp"], p["bigp"], p["sqp"], p["smallp"], p["rowp"], p["psp"]
            )
            mm1p = st.enter_context(tc.tile_pool(name="mm1io", bufs=1))
            ep = st.enter_context(tc.tile_pool(name="epool", bufs=2))

            ident, ones_col, ones_row, eps_col = _consts(nc, cp)

            lnw = cp.tile([P, DT], f32, name="lnw", tag="lnw")
            nc.sync.dma_start(lnw[:], lnw_d.rearrange("(t p) -> p t", p=P))
            lnb = cp.tile([P, DT], f32, name="lnb", tag="lnb")
            nc.sync.dma_start(lnb[:], lnb_d.rearrange("(t p) -> p t", p=P))

            # gate_av -> [P, 1] via K=1 broadcast matmul (f32, N=2)
            g2 = cp.tile([1, 2], f32, name="g2", tag="g2")
            nc.gpsimd.memset(g2[:], 0.0)
            nc.sync.dma_start(g2[:, 0:1], gav_d[None, :])
            psG = psp.tile([P, 2], f32, tag="ps2b", bufs=2, name="psG")
            nc.tensor.matmul(psG[:], ones_row[:].bitcast(f32), g2[:],
                             start=True, stop=True)
            gav_sb = cp.tile([P, 1], f32, name="gav_sb", tag="gav_sb")
            nc.scalar.copy(gav_sb[:], psG[:, 0:1])

            bn_slots = cp.tile([P, DT, BL], f32, name="bn_slots", tag="bn_slots")
            bn2_slots = cp.tile([P, DT, BL], f32, name="bn2_slots",
                                tag="bn2_slots")
            if PHASES == "AB" or not DO_RELU_ACC:
                nc.gpsimd.memset(bn_slots[:], 0.0)
            if PHASES == "AB" or not DO_TTR:
                nc.gpsimd.memset(bn2_slots[:], 0.0)

            for b in range(BL):
                with nc.named_scope(f"phaseABC_b{b}"):
                    x_nat = bigp.tile([P, DT, L], f32r, tag="xnat", bufs=2)
                    y_nat = bigp.tile([P, TT, L], f32r, tag="ynat")
                    xv = x_d[b].rearrange("(dt p) l -> p dt l", p=P).bitcast(f32r)
                    yv = y_d[b].rearrange("(tt p) l -> p tt l", p=P).bitcast(f32r)
                    for dt in range(DT):
                        nc.sync.dma_start(x_nat[:, dt], xv[:, dt])
                    for tt in range(TT):
                        nc.sync.dma_start(y_nat[:, tt], yv[:, tt])

                    # phase A: build xT, yT via PE transposes
                    xT = mm1p.tile([P, LT, D], f32r, tag="xT")
                    yT = mm1p.tile([P, LT, T], f32r, tag="yT")
                    for lt in range(LT):
                        lsl = slice(lt * P, (lt + 1) * P)
                        psA = psp.tile([P, D], f32r, tag="ps2b", bufs=2,
                                       name="psA")
                        for dt in range(DT):
                            nc.tensor.transpose(
                                psA[:, dt * P : (dt + 1) * P],
                                x_nat[:, dt, lsl], ident[:],
                            )
                        nc.scalar.copy(xT[:, lt], psA[:].bitcast(f32))
                        psB_ = psp.tile([P, T], f32r, tag="ps2b", bufs=2,
                                        name="psB_")
                        for tt in range(TT):
                            nc.tensor.transpose(
                                psB_[:, tt * P : (tt + 1) * P],
                                y_nat[:, tt, lsl], ident[:],
                            )
                        nc.scalar.copy(yT[:, lt], psB_[:].bitcast(f32))

                    # phase B: mm1 + softmax + E-transpose
                    ET = mm1p.tile([P, TT, D], f32r, tag="ET")
                    for dt in range(DT):
                        dsl = slice(dt * P, (dt + 1) * P)
                        psS = psp.tile([P, 2, 512], f32, tag="ps2b", bufs=2,
                                       name="psS")
                        for ch in range(2):
                            nsl = slice(ch * N1, (ch + 1) * N1)
                            for lt in range(LT):
                                nc.tensor.matmul(
                                    psS[:, ch, :N1], xT[:, lt, dsl],
                                    yT[:, lt, nsl], start=(lt == 0),
                                    stop=(lt == LT - 1),
                                )
                        negmax = smallp.tile([P, 1], f32, tag="negmax")
                        nc.vector.tensor_reduce(
                            negmax[:], psS[:, :, :N1],
                            axis=mybir.AxisListType.XY, op=OP.max, negate=True,
                        )
                        E = ep.tile([P, T], f32, tag="E")
                        sumexp = smallp.tile([P, 1], f32, tag="sumexp")
                        nc.scalar.activation(
                            E[:].rearrange("p (c n) -> p c n", c=2),
                            psS[:, :, :N1], AF.Exp, bias=negmax[:],
                            accum_out=sumexp[:],
                        )
                        rg = smallp.tile([P, 1], f32, tag="rg")
                        nc.vector.reciprocal(rg[:], sumexp[:])
                        nc.vector.tensor_scalar_mul(rg[:], rg[:], gav_sb[:])
                        Epr = ep.tile([P, T], f32r, tag="Epr")
                        nc.vector.tensor_scalar_mul(Epr[:], E[:], rg[:])
                        psE = psp.tile([P, D], f32r, tag="ps2b", bufs=2,
                                       name="psE")
                        for tt in range(TT):
                            csl = slice(tt * P, (tt + 1) * P)
                            nc.tensor.transpose(psE[:, csl], Epr[:, csl],
                                                ident[:])
                        nc.scalar.copy(
                            ET[:, :, dsl],
                            psE[:].rearrange("p (t c) -> p t c", c=P)
                            .bitcast(f32),
                        )

                    if PHASES == "AB":
                        # debug: dump ET into hr output, skip phase C
                        hrv_dbg = hr_d[b].rearrange("(dt p) l -> p dt l", p=P)
                        for tt in range(TT):
                            nc.sync.dma_start(
                                hrv_dbg[:, tt, 0:D], ET[:, tt].bitcast(f32)
                            )
                        continue
                    # phase C: mm2, h, LN(before), ReLU, BN partials, spill
                    y_nat2 = bigp.tile([P, TT, L], f32r, tag="ynat")
                    for tt in range(TT):
                        nc.sync.dma_start(y_nat2[:, tt], yv[:, tt])
                    psLsum = psLsq = None
                    if DO_STATS:
                        psLsum = psp.tile([1, 2, 512], f32, tag="psLsum")
                        psLsq = psp.tile([1, 2, 512], f32, tag="psLsq")
                    for dt in range(DT):
                        dsl = slice(dt * P, (dt + 1) * P)
                        psR = psp.tile([P, 2, 512], f32, tag="ps2b", bufs=2,
                                       name="psR")
                        for ch in range(2):
                            csl = slice(ch * N2, (ch + 1) * N2)
                            for tt in range(TT):
                                nc.tensor.matmul(
                                    psR[:, ch, :], ET[:, tt, dsl],
                                    y_nat2[:, tt, csl], start=(tt == 0),
                                    stop=(tt == TT - 1),
                                )
                        hv3 = x_nat[:, dt].rearrange("p (c n) -> p c n", c=2)
                        nc.vector.tensor_tensor(hv3, hv3.bitcast(f32), psR[:],
                                                OP.add)
                        if DO_STATS:
                            sq = sqp.tile([P, L], f32r, tag="sq")
                            nc.scalar.activation(
                                sq[:], x_nat[:, dt].bitcast(f32), AF.Square
                            )
                            for ch in range(2):
                                csl = slice(ch * N2, (ch + 1) * N2)
                                nc.tensor.matmul(
                                    psLsum[:1, ch, :], ones_col[:],
                                    x_nat[:, dt, csl],
                                    start=(dt == 0), stop=(dt == DT - 1),
                                )
                                nc.tensor.matmul(
                                    psLsq[:1, ch, :], ones_col[:], sq[:, csl],
                                    start=(dt == 0), stop=(dt == DT - 1),
                                )

                    if DO_STATS:
                        mu_row, rs_row = _ln_rows(nc, rowp, psLsum, psLsq,
                                                  eps_col)
                    else:
                        mu_f = rowp.tile([1, L], f32, tag="m2_row")
                        nc.gpsimd.memset(mu_f[:], 0.0)
                        mu_row = rowp.tile([1, L], f32r, tag="mu_row")
                        nc.vector.tensor_copy(mu_row[:], mu_f[:])
                        rs_f = rowp.tile([1, L], f32, tag="v_row")
                        nc.gpsimd.memset(rs_f[:], 1.0)
                        rs_row = rowp.tile([1, L], f32r, tag="rs_row")
                        nc.vector.tensor_copy(rs_row[:], rs_f[:])
                    psBmu, psBrs = _bcast_rows(nc, psp, ones_row, mu_row, rs_row)

                    hrv = hr_d[b].rearrange("(dt p) l -> p dt l", p=P)
                    for dt in range(DT):
                        hv3 = x_nat[:, dt].rearrange("p (c n) -> p c n", c=2)
                        nc.vector.tensor_tensor(hv3, hv3.bitcast(f32),
                                                psBmu[:], OP.subtract)
                        nc.vector.tensor_tensor(hv3, hv3.bitcast(f32),
                                                psBrs[:], OP.mult)
                        if DO_RELU_ACC:
                            nc.scalar.activation(
                                x_nat[:, dt], x_nat[:, dt].bitcast(f32),
                                AF.Relu, scale=lnw[:, dt : dt + 1],
                                bias=lnb[:, dt : dt + 1],
                                accum_out=bn_slots[:, dt, b : b + 1],
                            )
                        else:
                            nc.scalar.activation(
                                x_nat[:, dt], x_nat[:, dt].bitcast(f32),
                                AF.Relu, scale=lnw[:, dt : dt + 1],
                                bias=lnb[:, dt : dt + 1],
                            )
                        if DO_TTR:
                            sq = sqp.tile([P, L], f32r, tag="sq")
                            nc.scalar.activation(
                                sq[:], x_nat[:, dt].bitcast(f32), AF.Square,
                                accum_out=bn2_slots[:, dt, b : b + 1],
                            )
                        nc.sync.dma_start(hrv[:, dt], x_nat[:, dt].bitcast(f32))

            # per-core BN partial sums -> [2, D] output
            bn_sum = cp.tile([P, DT], f32, name="bn_sum", tag="bn_sum")
            nc.vector.tensor_reduce(
                bn_sum[:], bn_slots[:], axis=mybir.AxisListType.X, op=OP.add
            )
            bn_sq = cp.tile([P, DT], f32, name="bn_sq", tag="bn_sq")
            nc.vector.tensor_reduce(
                bn_sq[:], bn2_slots[:], axis=mybir.AxisListType.X, op=OP.add
            )
            stv = st_d.rearrange("s (t p) -> p s t", p=P)
            nc.sync.dma_start(stv[:, 0], bn_sum[:])
            nc.sync.dma_start(stv[:, 1], bn_sq[:])

    nc.compile()
    return nc


# ============== program 2: BN + 1x1 conv + post-LN + gate ==============

def build_program2():
    nc = bacc.Bacc("TRN2", target_bir_lowering=False, debug=False,
                   num_devices=NCORES)
    hr_d = nc.dram_tensor("hr", [BL, D, L], f32, kind="ExternalInput").ap()
    stg_d = nc.dram_tensor("bn_glob", [2, D], f32, kind="ExternalInput").ap()
    g_d = nc.dram_tensor("gate", [1], f32, kind="ExternalInput").ap()
    bng_d = nc.dram_tensor("bn_gamma", [D], f32, kind="ExternalInput").ap()
    bnb_d = nc.dram_tensor("bn_beta", [D], f32, kind="ExternalInput").ap()
    w_d = nc.dram_tensor("mlp_w", [D, D], f32, kind="ExternalInput").ap()
    lpw_d = nc.dram_tensor("ln_post_w", [D], f32, kind="ExternalInput").ap()
    lpb_d = nc.dram_tensor("ln_post_b", [D], f32, kind="ExternalInput").ap()
    out_d = nc.dram_tensor("out", [BL, D, L], f32, kind="ExternalOutput").ap()

    with tile.TileContext(nc) as tc:
        with ExitStack() as st:
            p = _common_pools(st, tc)
            cp, bigp, sqp, smallp, rowp, psp = (
                p["cp"], p["bigp"], p["sqp"], p["smallp"], p["rowp"], p["psp"]
            )
            outp = st.enter_context(tc.tile_pool(name="outpool", bufs=2))

            ident, ones_col, ones_row, eps_col = _consts(nc, cp)

            def load_param(ap_d, pname):
                t = cp.tile([P, DT], f32, name=pname, tag=pname)
                nc.sync.dma_start(t[:], ap_d.rearrange("(t p) -> p t", p=P))
                return t

            bng = load_param(bng_d, "bng")
            bnb = load_param(bnb_d, "bnb")
            lpw = load_param(lpw_d, "lpw")
            lpb = load_param(lpb_d, "lpb")

            g2 = cp.tile([1, 2], f32, name="g2", tag="g2")
            nc.gpsimd.memset(g2[:], 0.0)
            nc.sync.dma_start(g2[:, 0:1], g_d[None, :])
            psG = psp.tile([P, 2], f32, tag="ps2b", bufs=2, name="psG")
            nc.tensor.matmul(psG[:], ones_row[:].bitcast(f32), g2[:],
                             start=True, stop=True)
            g_sb = cp.tile([P, 1], f32, name="g_sb", tag="g_sb")
            nc.scalar.copy(g_sb[:], psG[:, 0:1])

            w2g = cp.tile([P, DT], f32, name="w2g", tag="w2g")
            nc.vector.tensor_scalar_mul(w2g[:], lpw[:], g_sb[:])
            b2g = cp.tile([P, DT], f32, name="b2g", tag="b2g")
            nc.vector.tensor_scalar_mul(b2g[:], lpb[:], g_sb[:])

            # WT = mlp_w^T -> [P(d), DT, D(o)]
            WT = cp.tile([P, DT, D], f32r, name="WT", tag="WT")
            w_nat = bigp.tile([P, OT, D], f32r, tag="ynat")
            wv = w_d.rearrange("(ot p) d -> p ot d", p=P).bitcast(f32r)
            for ot in range(OT):
                nc.sync.dma_start(w_nat[:, ot], wv[:, ot])
            for dt in range(DT):
                psW = psp.tile([P, D], f32r, tag="ps2b", bufs=2, name="psW")
                for ot in range(OT):
                    nc.tensor.transpose(
                        psW[:, ot * P : (ot + 1) * P],
                        w_nat[:, ot, dt * P : (dt + 1) * P], ident[:],
                    )
                nc.scalar.copy(WT[:, dt], psW[:].bitcast(f32))

            # BN finalize: s_bn = gamma * rsqrt(var+eps); q = beta - s_bn*mu
            gstats = cp.tile([P, 2, DT], f32, name="gstats", tag="gstats")
            nc.sync.dma_start(gstats[:],
                              stg_d.rearrange("s (t p) -> p s t", p=P))
            mu_bn = cp.tile([P, DT], f32, name="mu_bn", tag="mu_bn")
            nc.vector.tensor_scalar_mul(mu_bn[:], gstats[:, 0], 1.0 / NB)
            ex2 = cp.tile([P, DT], f32, name="ex2", tag="ex2")
            nc.vector.tensor_scalar_mul(ex2[:], gstats[:, 1], 1.0 / NB)
            mu2 = cp.tile([P, DT], f32, name="mu2", tag="mu2")
            nc.vector.tensor_tensor(mu2[:], mu_bn[:], mu_bn[:], OP.mult)
            var_bn = cp.tile([P, DT], f32, name="var_bn", tag="var_bn")
            nc.vector.tensor_tensor(var_bn[:], ex2[:], mu2[:], OP.subtract)
            lv = cp.tile([P, DT], f32, name="lv", tag="lv")
            nc.scalar.activation(lv[:], var_bn[:], AF.Ln, bias=eps_col[:])
            rs_bn = cp.tile([P, DT], f32, name="rs_bn", tag="rs_bn")
            nc.scalar.activation(rs_bn[:], lv[:], AF.Exp, scale=-0.5)
            s_bn = cp.tile([P, DT], f32, name="s_bn", tag="s_bn")
            nc.vector.tensor_tensor(s_bn[:], bng[:], rs_bn[:], OP.mult)
            smu = cp.tile([P, DT], f32, name="smu", tag="smu")
            nc.vector.tensor_tensor(smu[:], s_bn[:], mu_bn[:], OP.mult)
            q = cp.tile([P, DT], f32r, name="q", tag="q")
            nc.vector.tensor_tensor(q[:], bnb[:], smu[:], OP.subtract)

            # cvec[o] = sum_d W[o,d] * q[d]  (f32, N=1)
            psC = psp.tile([P, OT], f32, tag="ps2b", bufs=2, name="psC")
            for ot in range(OT):
                for dt in range(DT):
                    nc.tensor.matmul(
                        psC[:, ot : ot + 1],
                        WT[:, dt, ot * P : (ot + 1) * P].bitcast(f32),
                        q[:, dt : dt + 1].bitcast(f32),
                        start=(dt == 0), stop=(dt == DT - 1),
                    )
            cvec = cp.tile([P, OT], f32, name="cvec", tag="cvec")
            nc.scalar.copy(cvec[:], psC[:])

            for dt in range(DT):
                nc.vector.tensor_scalar_mul(
                    WT[:, dt], WT[:, dt].bitcast(f32), s_bn[:, dt : dt + 1]
                )

            for b in range(BL):
                with nc.named_scope(f"phaseD_b{b}"):
                    hr = bigp.tile([P, DT, L], f32r, tag="xnat", bufs=2)
                    hrv = hr_d[b].rearrange("(dt p) l -> p dt l", p=P)
                    for dt in range(DT):
                        nc.sync.dma_start(hr[:, dt], hrv[:, dt].bitcast(f32r))
                    z = bigp.tile([P, OT, L], f32r, tag="ynat")

                    psPsum = psp.tile([1, 2, 512], f32, tag="psLsum")
                    psPsq = psp.tile([1, 2, 512], f32, tag="psLsq")
                    for ot in range(OT):
                        osl = slice(ot * P, (ot + 1) * P)
                        psZ = psp.tile([P, 2, 512], f32, tag="ps2b", bufs=2,
                                       name="psZ")
                        for ch in range(2):
                            csl = slice(ch * N2, (ch + 1) * N2)
                            for dt in range(DT):
                                nc.tensor.matmul(
                                    psZ[:, ch, :], WT[:, dt, osl],
                                    hr[:, dt, csl], start=(dt == 0),
                                    stop=(dt == DT - 1),
                                )
                        nc.scalar.activation(
                            z[:, ot].rearrange("p (c n) -> p c n", c=2),
                            psZ[:], AF.Identity, bias=cvec[:, ot : ot + 1],
                        )
                        sq = sqp.tile([P, L], f32r, tag="sq")
                        nc.scalar.activation(sq[:], z[:, ot].bitcast(f32),
                                             AF.Square)
                        for ch in range(2):
                            csl = slice(ch * N2, (ch + 1) * N2)
                            nc.tensor.matmul(
                                psPsum[:1, ch, :], ones_col[:], z[:, ot, csl],
                                start=(ot == 0), stop=(ot == OT - 1),
                            )
                            nc.tensor.matmul(
                                psPsq[:1, ch, :], ones_col[:], sq[:, csl],
                                start=(ot == 0), stop=(ot == OT - 1),
                            )

                    mu_row, rs_row = _ln_rows(nc, rowp, psPsum, psPsq, eps_col)
                    psBmu, psBrs = _bcast_rows(nc, psp, ones_row, mu_row,
                                               rs_row)

                    ov = out_d[b].rearrange("(ot p) l -> p ot l", p=P)
                    for ot in range(OT):
                        zv3 = z[:, ot].rearrange("p (c n) -> p c n", c=2)
                        nc.vector.tensor_tensor(zv3, zv3.bitcast(f32),
                                                psBmu[:], OP.subtract)
                        nc.vector.tensor_tensor(zv3, zv3.bitcast(f32),
                                                psBrs[:], OP.mult)
                        osb = outp.tile([P, L], f32, tag="osb")
                        nc.vector.tensor_scalar(
                            osb[:], z[:, ot].bitcast(f32),
                            w2g[:, ot : ot + 1], b2g[:, ot : ot + 1],
                            op0=OP.mult, op1=OP.add,
                        )
                        nc.sync.dma_start(ov[:, ot], osb[:])

    nc.compile()
    return nc


_PROGRAMS = None


def _get_programs():
    global _PROGRAMS
    if _PROGRAMS is None:
        _PROGRAMS = (build_program1(), build_program2())
    return _PROGRAMS


def run_staged(x, y, params, trace=False):
    """Run both programs; returns (out [B,D,L], exec_ns_total, scopes)."""
    nc1, nc2 = _get_programs()
    in1 = []
    for c in range(NCORES):
        in1.append({
            "x": x[c * BL : (c + 1) * BL],
            "y": y[c * BL : (c + 1) * BL],
            "gate_av": params["gate_av"],
            "ln_before_w": params["ln_before_w"],
            "ln_before_b": params["ln_before_b"],
        })
    r1 = run_bass_kernel_spmd(nc1, in1, core_ids=list(range(NCORES)),
                              trace=trace)
    bn_glob = np.sum(
        [r1.results[c]["bn_part"] for c in range(NCORES)], axis=0,
        dtype=np.float32,
    ).astype(np.float32)
    in2 = []
    for c in range(NCORES):
        in2.append({
            "hr": r1.results[c]["hr"],
            "bn_glob": bn_glob,
            "gate": params["gate"],
            "bn_gamma": params["bn_gamma"],
            "bn_beta": params["bn_beta"],
            "mlp_w": params["mlp_w"],
            "ln_post_w": params["ln_post_w"],
            "ln_post_b": params["ln_post_b"],
        })
    r2 = run_bass_kernel_spmd(nc2, in2, core_ids=list(range(NCORES)),
                              trace=trace)
    out = np.concatenate([r2.results[c]["out"] for c in range(NCORES)], axis=0)
    exec_ns = None
    if r1.exec_time_ns is not None and r2.exec_time_ns is not None:
        exec_ns = r1.exec_time_ns + r2.exec_time_ns
    scopes = {}
    for r in (r1, r2):
        if r.per_core_scope_times:
            scopes.update(r.per_core_scope_times)
    return out, exec_ns, scopes


def kernel(**inputs) -> np.ndarray:
    import concourse.bass_utils as bu

    bu.upload_artifacts = lambda d: d  # no artifact store in container

    x = np.ascontiguousarray(np.asarray(inputs["x"])[..., 0], dtype=np.float32)
    y = np.ascontiguousarray(np.asarray(inputs["y"])[..., 0], dtype=np.float32)
    params = {
        k: np.ascontiguousarray(np.asarray(inputs[k]), dtype=np.float32)
        for k in [
            "gate_av", "gate", "ln_before_w", "ln_before_b", "bn_gamma",
            "bn_beta", "mlp_w", "ln_post_w", "ln_post_b",
        ]
    }
    out, _, _ = run_staged(x, y, params, trace=False)
    return out[..., None]

